# revision 13
# baseline (speedup 1.0000x reference)
"""Trainium2 Bass kernel for nn_ChunkedQuantHead.

Computation (see reference):
  xc   = x.reshape(B, 16, 256)
  acts = mean(|xc|, axis=(0, 2))           # global per-chunk stat
  top4 = top_k(acts, 4)                    # global chunk routing
  routed = einsum('bkc,koc->bo', xc[:, top4], expert_w[top4]) + expert_b
  w_eff  = quant_w if max(acts) > 0.5 else sign(quant_w)*mean|quant_w|
  out    = routed @ w_eff.T + quant_b

Strategy (8 cores, data-parallel over batch):
  - Each core streams its 2048x4096 f32 shard from HBM ONCE (bf16 cast
    during the SWDGE DMA).  In the same pass it computes per-chunk
    |x| partial sums (DVE fused abs+reduce) AND projects ALL 16 chunks
    through BOTH quantized-head variants at once: the head weights are
    folded into the expert weights host-side
      w2[c, 0, o', f] = sum_o wbin[o', o] w[c, o, f]      (1-bit head)
      w2[c, 1, o', f] = sum_o (qw - wbin)[o', o] w[c, o, f]  (delta)
    so the PE emits z[b, (c, v, o')] per tile and no per-sample work
    remains after the routing mask is known:
      out[b, o'] = sum_c mask_c * (z[b,c,0,o'] + cond * z[b,c,1,o'])
                 + bias(cond)
  - ONE tiny AllReduce combines the per-core chunk stats right after
    the last tile's stats land (no serialized collective pair); the
    top-4 selection is mask-based (no dynamic control flow).
  - The tail is a single fused DVE multiply + contiguous reduce: the
    (cv, o) -> (o, cv) layout swap happens inside the hidden per-tile
    PSUM->SBUF copy so the reduction axis is stride-1.
"""

import numpy as np

import concourse.bacc as bacc
import concourse.tile as tile
import concourse.mybir as mybir
from concourse.bass_utils import run_bass_kernel_spmd

F32 = mybir.dt.float32
BF16 = mybir.dt.bfloat16
AX = mybir.AxisListType
OP = mybir.AluOpType

N_CORES = 8
B, F = 16384, 4096
CHUNKS, CHUNK, OUT = 16, 256, 10
TOPK = 4
THRESH = 0.5
BC = B // N_CORES            # 2048 rows per core
P = 128
TILES = BC // P              # 16 tiles of 128 rows
NV = 2                       # head variants: v=0 bin, v=1 delta
CV = CHUNKS * NV             # 32
W2C = CHUNKS * NV * OUT      # 320 z-columns per tile
SUM_THRESH = THRESH * B * CHUNK  # compare sum(|x|) against this (scale folded)
BIG_NEG = -1.0e30

_CACHE = {}


def _build():
    nc = bacc.Bacc(
        "TRN2",
        target_bir_lowering=False,
        debug=False,
        num_devices=N_CORES,
    )

    x_d = nc.dram_tensor("x", [BC, F], F32, kind="ExternalInput")
    idb_d = nc.dram_tensor("id_bf", [P, P], BF16, kind="ExternalInput")
    # folded expert+head weights, pre-arranged host-side:
    #   w2_sb[p, h*320 + c*20 + v*10 + o] = w2[c, v, o, h*128 + p]  (bf16)
    w_d = nc.dram_tensor("w2_sb", [P, 2 * W2C], BF16, kind="ExternalInput")
    bb_d = nc.dram_tensor("bias_b0", [1, OUT], F32, kind="ExternalInput")
    bd_d = nc.dram_tensor("bias_d", [1, OUT], F32, kind="ExternalInput")
    out_d = nc.dram_tensor("out", [BC, OUT], F32, kind="ExternalOutput")

    with tile.TileContext(nc) as tc:
        with (
            tc.tile_pool(name="const", bufs=1) as constp,
            tc.tile_pool(name="persist", bufs=1) as perp,
            tc.tile_pool(name="xb", bufs=8) as xbp,
            tc.tile_pool(name="xt", bufs=2) as xtp,
            tc.tile_pool(name="tail", bufs=1) as tailp,
            tc.tile_pool(name="ps_misc", bufs=2, space="PSUM") as psm,
            tc.tile_pool(name="dram", bufs=1, space="DRAM") as dramp,
        ):
            # ---- constants ----
            id_bf = constp.tile([P, P], BF16)
            nc.sync.dma_start(id_bf[:, :], idb_d.ap())
            w2_sb = constp.tile([P, 2 * W2C], BF16)
            nc.sync.dma_start(w2_sb[:, :], w_d.ap())
            bb_row = constp.tile([1, OUT], F32)
            nc.sync.dma_start(bb_row[:, :], bb_d.ap())
            bd_row = constp.tile([1, OUT], F32)
            nc.sync.dma_start(bd_row[:, :], bd_d.ap())
            ones_col = constp.tile([P, 1], F32)
            nc.vector.memset(ones_col[:, :], 1.0)
            ones_row = constp.tile([1, P], F32)
            nc.vector.memset(ones_row[:, :], 1.0)

            # persistent accumulators
            # z_all[p, t*320 + o*32 + (2c+v)]  (o-major, cv contiguous)
            z_all = perp.tile([P, TILES * W2C], BF16)
            red_all = perp.tile([P, TILES * CHUNKS], F32)       # [128, 256]

            # DRAM bounce buffers for the AllReduce
            cc_in = dramp.tile([1, CHUNKS], F32)
            cc_out = dramp.tile([1, CHUNKS], F32, addr_space="Shared")
            # dummy collective: pays the ~11us one-time CC-core startup
            # early so the real AllReduce dispatches in ~1us everywhere
            warm_in = dramp.tile([1, 1], F32)
            warm_out = dramp.tile([1, 1], F32, addr_space="Shared")

            # ---- main pass over x: stats + all-chunk two-variant proj ----
            with (
                tc.tile_pool(name="ps_tr", bufs=2, space="PSUM") as pstr,
                tc.tile_pool(name="ps_y", bufs=2, space="PSUM") as psy,
            ):
                warm_sb = tailp.tile([1, 1], F32, tag="warm_sb")
                nc.vector.memset(warm_sb[:, :], 0.0)
                nc.sync.dma_start(warm_in[:, :], warm_sb[:, :])
                nc.gpsimd.collective_compute(
                    "AllReduce",
                    OP.add,
                    replica_groups=[list(range(N_CORES))],
                    ins=[warm_in.opt()],
                    outs=[warm_out.opt()],
                )

                for t in range(TILES):
                    xb = xbp.tile([P, F], BF16, tag="xb")
                    # SWDGE DMA with f32 -> bf16 cast in the datapath;
                    # the last tile lands in halves so its stats reduce
                    # (critical path into the AllReduce) is half as long
                    if t == TILES - 1:
                        nc.gpsimd.dma_start(
                            xb[:, 0:F // 2],
                            x_d.ap()[t * P:(t + 1) * P, 0:F // 2],
                        )
                        nc.vector.tensor_reduce(
                            red_all[:, t * CHUNKS:t * CHUNKS + CHUNKS // 2],
                            xb[:, 0:F // 2].rearrange(
                                "p (c f) -> p c f", f=CHUNK
                            ),
                            axis=AX.X,
                            op=OP.add,
                            apply_absolute_value=True,
                        )
                        nc.gpsimd.dma_start(
                            xb[:, F // 2:F],
                            x_d.ap()[t * P:(t + 1) * P, F // 2:F],
                        )
                        nc.vector.tensor_reduce(
                            red_all[:, t * CHUNKS + CHUNKS // 2:
                                    (t + 1) * CHUNKS],
                            xb[:, F // 2:F].rearrange(
                                "p (c f) -> p c f", f=CHUNK
                            ),
                            axis=AX.X,
                            op=OP.add,
                            apply_absolute_value=True,
                        )
                    else:
                        nc.gpsimd.dma_start(
                            xb[:, :], x_d.ap()[t * P:(t + 1) * P, :]
                        )
                        # per-chunk sum of |x| (fused abs+reduce)
                        nc.vector.tensor_reduce(
                            red_all[:, t * CHUNKS:(t + 1) * CHUNKS],
                            xb[:, :].rearrange("p (c f) -> p c f", f=CHUNK),
                            axis=AX.X,
                            op=OP.add,
                            apply_absolute_value=True,
                        )

                    if t == TILES - 1:
                        # stats for all tiles are in: partition-reduce and
                        # kick the single AllReduce before tile 15's PE work
                        acts_p = tailp.tile([P, CHUNKS], F32, tag="acts_p")
                        nc.vector.tensor_reduce(
                            acts_p[:, :],
                            red_all[:, :].rearrange(
                                "p (t c) -> p c t", c=CHUNKS
                            ),
                            axis=AX.X,
                            op=OP.add,
                        )
                        ps_a = psm.tile([1, CHUNKS], F32, tag="psmisc")
                        nc.tensor.matmul(
                            ps_a[:, :], lhsT=ones_col[:, :], rhs=acts_p[:, :],
                            start=True, stop=True,
                        )
                        cc_sb = tailp.tile([1, CHUNKS], F32, tag="cc_sb")
                        nc.scalar.copy(cc_sb[:, :], ps_a[:, :])
                        nc.sync.dma_start(cc_in[:, :], cc_sb[:, :])
                        nc.gpsimd.collective_compute(
                            "AllReduce",
                            OP.add,
                            replica_groups=[list(range(N_CORES))],
                            ins=[cc_in.opt()],
                            outs=[cc_out.opt()],
                        )

                    # transpose all 32 [128,128] blocks: x[b, f] -> xT[f, b]
                    xt = xtp.tile([P, F], BF16, tag="xt")
                    for g in range(2):
                        ps = pstr.tile([P, 16 * P], BF16, tag="ps_tr")
                        for j in range(16):
                            k = 16 * g + j
                            nc.tensor.transpose(
                                ps[:, j * P:(j + 1) * P],
                                xb[:, k * P:(k + 1) * P],
                                id_bf[:, :],
                            )
                        nc.scalar.copy(
                            xt[:, g * 16 * P:(g + 1) * 16 * P], ps[:, :]
                        )

                    # project every chunk through both head variants:
                    # psy_t[b, cv*10 + o] accumulated over the 2 halves
                    psy_t = psy.tile([P, W2C], F32, tag="psy")
                    for c in range(CHUNKS):
                        for h in range(2):
                            kh = 2 * c + h
                            nc.tensor.matmul(
                                psy_t[:, c * NV * OUT:(c + 1) * NV * OUT],
                                lhsT=xt[:, kh * P:(kh + 1) * P],
                                rhs=w2_sb[:, h * W2C + c * NV * OUT:
                                          h * W2C + (c + 1) * NV * OUT],
                                start=(h == 0),
                                stop=(h == 1),
                            )
                    # PSUM -> SBUF with (cv, o) -> (o, cv) layout swap so the
                    # tail reduce runs on a contiguous axis; f32 -> bf16 cast
                    nc.scalar.copy(
                        z_all[:, t * W2C:(t + 1) * W2C].rearrange(
                            "p (o cv) -> p o cv", cv=CV
                        ),
                        psy_t[:, :].rearrange("p (cv o) -> p o cv", o=OUT),
                    )

            # ---- tail: S -> top-4 mask -> fused combine -> store ----
            S = tailp.tile([1, CHUNKS], F32, tag="S")
            nc.sync.dma_start(S[:, :], cc_out[:, :])

            # top-4 threshold via 4x (max + mask-out); all on partition 0
            cur = tailp.tile([1, CHUNKS], F32, tag="cur")
            src = S
            m1 = None
            mk = None
            for k in range(TOPK):
                mk = tailp.tile([1, 1], F32, tag=f"mk{k}")
                nc.vector.tensor_reduce(mk[:, :], src[:, :], axis=AX.X, op=OP.max)
                if k == 0:
                    m1 = mk
                if k < TOPK - 1:
                    sel = tailp.tile([1, CHUNKS], F32, tag="sel")
                    # sel = (src >= mk) * BIG_NEG  in one fused op
                    nc.vector.tensor_scalar(
                        sel[:, :], src[:, :], mk[:, :], BIG_NEG,
                        op0=OP.is_ge, op1=OP.mult,
                    )
                    nc.vector.tensor_tensor(cur[:, :], src[:, :], sel[:, :], op=OP.add)
                    src = cur
            m4 = mk  # 4th largest

            cond = tailp.tile([1, 1], F32, tag="cond")
            nc.vector.tensor_scalar(
                cond[:, :], m1[:, :], float(SUM_THRESH), None, op0=OP.is_gt
            )

            # combine row: [0:32) per-(c,v) weights, [32:42) bias(cond)
            BROW = CV + OUT
            brow = tailp.tile([1, BROW], F32, tag="brow")
            brow_cv = brow[:, 0:CV].rearrange("a (c v) -> a c v", v=NV)
            nc.vector.tensor_scalar(
                brow_cv[:, :, 0:1], S[:, :].unsqueeze(2), m4[:, :],
                None, op0=OP.is_ge,
            )
            nc.vector.tensor_scalar(
                brow_cv[:, :, 1:2], brow_cv[:, :, 0:1], cond[:, :],
                None, op0=OP.mult,
            )
            nc.vector.tensor_scalar(
                brow[:, CV:BROW], bd_row[:, :], cond[:, :], None, op0=OP.mult
            )
            nc.vector.tensor_tensor(
                brow[:, CV:BROW], brow[:, CV:BROW], bb_row[:, :], op=OP.add
            )

            # broadcast row -> all 128 partitions via K=1 matmul
            ps_b = psm.tile([P, BROW], F32, tag="psmisc")
            nc.tensor.matmul(
                ps_b[:, :], lhsT=ones_row[:, :], rhs=brow[:, :],
                start=True, stop=True,
            )
            bc32 = tailp.tile([P, CV], BF16, tag="bc32")
            nc.scalar.copy(bc32[:, :], ps_b[:, 0:CV])
            bias_bc = tailp.tile([P, OUT], F32, tag="bias_bc")
            nc.scalar.copy(bias_bc[:, :], ps_b[:, CV:BROW])

            # fused masked combine (DVE runs ~0.93 cyc/elem with bf16
            # outputs, ~1.5-1.8 with f32 outputs or reduces): bf16 mult,
            # one bf16 halving add, then a f32 reduce over the final 16
            tmp = tailp.tile([P, TILES * W2C], BF16, tag="tmp")
            nc.vector.tensor_tensor(
                tmp[:, :].rearrange("p (t o cv) -> p t o cv", o=OUT, cv=CV),
                z_all[:, :].rearrange("p (t o cv) -> p t o cv", o=OUT, cv=CV),
                bc32[:, :].unsqueeze(1).unsqueeze(2).broadcast_to(
                    [P, TILES, OUT, CV]
                ),
                op=OP.mult,
            )
            tv = tmp[:, :].rearrange("p (t o cv) -> p t o cv", o=OUT, cv=CV)
            nc.vector.tensor_tensor(
                tv[:, :, :, 0:CV // 2], tv[:, :, :, 0:CV // 2],
                tv[:, :, :, CV // 2:CV], op=OP.add,
            )
            out_sb = tailp.tile([P, TILES * OUT], F32, tag="out_sb")
            nc.vector.tensor_reduce(
                out_sb[:, :].rearrange("p (t o) -> p t o", o=OUT),
                tv[:, :, :, 0:CV // 2],
                axis=AX.X,
                op=OP.add,
            )
            nc.vector.tensor_tensor(
                out_sb[:, :].rearrange("p (t o) -> p t o", o=OUT),
                out_sb[:, :].rearrange("p (t o) -> p t o", o=OUT),
                bias_bc[:, :].unsqueeze(1).broadcast_to([P, TILES, OUT]),
                op=OP.add,
            )

            # store both halves in parallel on the two HWDGE rings
            half = TILES // 2
            nc.sync.dma_start(
                out_d.ap()[0:half * P, :].rearrange("(t p) o -> p t o", p=P),
                out_sb[:, 0:half * OUT].rearrange("p (t o) -> p t o", o=OUT),
            )
            nc.scalar.dma_start(
                out_d.ap()[half * P:TILES * P, :].rearrange(
                    "(t p) o -> p t o", p=P
                ),
                out_sb[:, half * OUT:TILES * OUT].rearrange(
                    "p (t o) -> p t o", o=OUT
                ),
            )

    nc.compile()
    return nc


def _get_nc():
    if "nc" not in _CACHE:
        _CACHE["nc"] = _build()
    return _CACHE["nc"]


def _prep_weights(expert_w, expert_b, quant_w, quant_b):
    import ml_dtypes

    qmean = np.float32(np.mean(np.abs(quant_w)))
    wbin = (np.sign(quant_w) * qmean).astype(np.float32)       # [o', o]
    d = (quant_w - wbin).astype(np.float32)
    # fold head variants into expert weights: w2[c, v, o', f]
    w2_bin = np.einsum("po,cof->cpf", wbin, expert_w)
    w2_d = np.einsum("po,cof->cpf", d, expert_w)
    w2 = np.stack([w2_bin, w2_d], axis=1)                      # [16, 2, 10, 256]
    wr = w2.reshape(CHUNKS, NV, OUT, 2, P)                     # c, v, o, h, p
    w2_sb = np.ascontiguousarray(
        wr.transpose(4, 3, 0, 1, 2).reshape(P, 2 * W2C)
    ).astype(ml_dtypes.bfloat16)
    bias_b0 = (expert_b @ wbin.T + quant_b).reshape(1, OUT).astype(np.float32)
    bias_d = (expert_b @ d.T).reshape(1, OUT).astype(np.float32)
    id_bf = np.eye(P, dtype=ml_dtypes.bfloat16)
    return w2_sb, bias_b0, bias_d, id_bf


def kernel(x, expert_w, expert_b, quant_w, quant_b):
    x = np.ascontiguousarray(np.asarray(x, dtype=np.float32))
    expert_w = np.asarray(expert_w, dtype=np.float32)
    expert_b = np.asarray(expert_b, dtype=np.float32)
    quant_w = np.asarray(quant_w, dtype=np.float32)
    quant_b = np.asarray(quant_b, dtype=np.float32)

    w2_sb, bias_b0, bias_d, id_bf = _prep_weights(
        expert_w, expert_b, quant_w, quant_b
    )
    nc = _get_nc()
    in_maps = []
    for i in range(N_CORES):
        in_maps.append({
            "x": np.ascontiguousarray(x[i * BC:(i + 1) * BC]),
            "w2_sb": w2_sb,
            "bias_b0": bias_b0,
            "bias_d": bias_d,
            "id_bf": id_bf,
        })

    res = run_bass_kernel_spmd(nc, in_maps, core_ids=list(range(N_CORES)))
    out = np.concatenate(
        [np.asarray(res.results[i]["out"]) for i in range(N_CORES)], axis=0
    )
    return out.astype(np.float32)


# revision 15
# speedup vs baseline: 1.0364x; 1.0364x over previous
"""Trainium2 Bass kernel for nn_ChunkedQuantHead.

Computation (see reference):
  xc   = x.reshape(B, 16, 256)
  acts = mean(|xc|, axis=(0, 2))           # global per-chunk stat
  top4 = top_k(acts, 4)                    # global chunk routing
  routed = einsum('bkc,koc->bo', xc[:, top4], expert_w[top4]) + expert_b
  w_eff  = quant_w if max(acts) > 0.5 else sign(quant_w)*mean|quant_w|
  out    = routed @ w_eff.T + quant_b

Strategy (8 cores, data-parallel over batch):
  - Each core streams its 2048x4096 f32 shard from HBM ONCE (bf16 cast
    during the SWDGE DMA).  In the same pass it computes per-chunk
    |x| partial sums (DVE fused abs+reduce) AND projects ALL 16 chunks
    through BOTH quantized-head variants at once: the head weights are
    folded into the expert weights host-side
      w2[c, 0, o', f] = sum_o wbin[o', o] w[c, o, f]      (1-bit head)
      w2[c, 1, o', f] = sum_o (qw - wbin)[o', o] w[c, o, f]  (delta)
    so the PE emits z[b, (c, v, o')] per tile and no per-sample work
    remains after the routing mask is known:
      out[b, o'] = sum_c mask_c * (z[b,c,0,o'] + cond * z[b,c,1,o'])
                 + bias(cond)
  - ONE tiny AllReduce combines the per-core chunk stats right after
    the last tile's stats land (no serialized collective pair); the
    top-4 selection is mask-based (no dynamic control flow).
  - The tail is a single fused DVE multiply + contiguous reduce: the
    (cv, o) -> (o, cv) layout swap happens inside the hidden per-tile
    PSUM->SBUF copy so the reduction axis is stride-1.
"""

import numpy as np

import concourse.bacc as bacc
import concourse.tile as tile
import concourse.mybir as mybir
from concourse.bass_utils import run_bass_kernel_spmd

F32 = mybir.dt.float32
BF16 = mybir.dt.bfloat16
AX = mybir.AxisListType
OP = mybir.AluOpType

N_CORES = 8
B, F = 16384, 4096
CHUNKS, CHUNK, OUT = 16, 256, 10
TOPK = 4
THRESH = 0.5
BC = B // N_CORES            # 2048 rows per core
P = 128
TILES = BC // P              # 16 tiles of 128 rows
NV = 2                       # head variants: v=0 bin, v=1 delta
CV = CHUNKS * NV             # 32
W2C = CHUNKS * NV * OUT      # 320 z-columns per tile
SUM_THRESH = THRESH * B * CHUNK  # compare sum(|x|) against this (scale folded)
BIG_NEG = -1.0e30

_CACHE = {}


def _build():
    nc = bacc.Bacc(
        "TRN2",
        target_bir_lowering=False,
        debug=False,
        num_devices=N_CORES,
    )

    x_d = nc.dram_tensor("x", [BC, F], F32, kind="ExternalInput")
    idb_d = nc.dram_tensor("id_bf", [P, P], BF16, kind="ExternalInput")
    # folded expert+head weights, pre-arranged host-side:
    #   w2_sb[p, h*320 + c*20 + v*10 + o] = w2[c, v, o, h*128 + p]  (bf16)
    w_d = nc.dram_tensor("w2_sb", [P, 2 * W2C], BF16, kind="ExternalInput")
    bb_d = nc.dram_tensor("bias_b0", [1, OUT], F32, kind="ExternalInput")
    bd_d = nc.dram_tensor("bias_d", [1, OUT], F32, kind="ExternalInput")
    out_d = nc.dram_tensor("out", [BC, OUT], F32, kind="ExternalOutput")

    with tile.TileContext(nc) as tc:
        with (
            tc.tile_pool(name="const", bufs=1) as constp,
            tc.tile_pool(name="persist", bufs=1) as perp,
            tc.tile_pool(name="xb", bufs=6) as xbp,
            tc.tile_pool(name="xt", bufs=2) as xtp,
            tc.tile_pool(name="tail", bufs=1) as tailp,
            tc.tile_pool(name="ps_misc", bufs=2, space="PSUM") as psm,
            tc.tile_pool(name="dram", bufs=1, space="DRAM") as dramp,
        ):
            # ---- constants ----
            id_bf = constp.tile([P, P], BF16)
            nc.sync.dma_start(id_bf[:, :], idb_d.ap())
            w2_sb = constp.tile([P, 2 * W2C], BF16)
            nc.sync.dma_start(w2_sb[:, :], w_d.ap())
            bb_row = constp.tile([1, OUT], F32)
            nc.sync.dma_start(bb_row[:, :], bb_d.ap())
            bd_row = constp.tile([1, OUT], F32)
            nc.sync.dma_start(bd_row[:, :], bd_d.ap())
            ones_col = constp.tile([P, 1], F32)
            nc.vector.memset(ones_col[:, :], 1.0)
            ones_row = constp.tile([1, P], F32)
            nc.vector.memset(ones_row[:, :], 1.0)

            # persistent accumulators
            # z_all[p, t*320 + o*32 + (2c+v)]  (o-major, cv contiguous)
            z_all = perp.tile([P, TILES * W2C], BF16)
            red_all = perp.tile([P, TILES * CHUNKS], F32)       # [128, 256]

            # DRAM bounce buffers for the AllReduce
            cc_in = dramp.tile([1, CHUNKS], F32)
            cc_out = dramp.tile([1, CHUNKS], F32, addr_space="Shared")
            # dummy collective: pays the ~11us one-time CC-core startup
            # early so the real AllReduce dispatches in ~1us everywhere
            warm_in = dramp.tile([1, 1], F32)
            warm_out = dramp.tile([1, 1], F32, addr_space="Shared")

            # ---- main pass over x: stats + all-chunk two-variant proj ----
            with (
                tc.tile_pool(name="ps_tr", bufs=2, space="PSUM") as pstr,
                tc.tile_pool(name="ps_y", bufs=2, space="PSUM") as psy,
            ):
                warm_sb = tailp.tile([1, 1], F32, tag="warm_sb")
                nc.vector.memset(warm_sb[:, :], 0.0)
                nc.sync.dma_start(warm_in[:, :], warm_sb[:, :])
                nc.gpsimd.collective_compute(
                    "AllReduce",
                    OP.add,
                    replica_groups=[list(range(N_CORES))],
                    ins=[warm_in.opt()],
                    outs=[warm_out.opt()],
                )

                for t in range(TILES):
                    xb = xbp.tile([P, F], BF16, tag="xb")
                    # SWDGE DMA with f32 -> bf16 cast in the datapath;
                    # the last tile lands in halves so its stats reduce
                    # (critical path into the AllReduce) is half as long
                    if t == TILES - 1:
                        nc.gpsimd.dma_start(
                            xb[:, 0:F // 2],
                            x_d.ap()[t * P:(t + 1) * P, 0:F // 2],
                        )
                        nc.vector.tensor_reduce(
                            red_all[:, t * CHUNKS:t * CHUNKS + CHUNKS // 2],
                            xb[:, 0:F // 2].rearrange(
                                "p (c f) -> p c f", f=CHUNK
                            ),
                            axis=AX.X,
                            op=OP.add,
                            apply_absolute_value=True,
                        )
                        nc.gpsimd.dma_start(
                            xb[:, F // 2:F],
                            x_d.ap()[t * P:(t + 1) * P, F // 2:F],
                        )
                        nc.vector.tensor_reduce(
                            red_all[:, t * CHUNKS + CHUNKS // 2:
                                    (t + 1) * CHUNKS],
                            xb[:, F // 2:F].rearrange(
                                "p (c f) -> p c f", f=CHUNK
                            ),
                            axis=AX.X,
                            op=OP.add,
                            apply_absolute_value=True,
                        )
                    else:
                        nc.gpsimd.dma_start(
                            xb[:, :], x_d.ap()[t * P:(t + 1) * P, :]
                        )
                        # per-chunk sum of |x| (fused abs+reduce)
                        nc.vector.tensor_reduce(
                            red_all[:, t * CHUNKS:(t + 1) * CHUNKS],
                            xb[:, :].rearrange("p (c f) -> p c f", f=CHUNK),
                            axis=AX.X,
                            op=OP.add,
                            apply_absolute_value=True,
                        )

                    if t == TILES - 1:
                        # stats for all tiles are in: partition-reduce and
                        # kick the single AllReduce before tile 15's PE work
                        acts_p = tailp.tile([P, CHUNKS], F32, tag="acts_p")
                        nc.vector.tensor_reduce(
                            acts_p[:, :],
                            red_all[:, :].rearrange(
                                "p (t c) -> p c t", c=CHUNKS
                            ),
                            axis=AX.X,
                            op=OP.add,
                        )
                        ps_a = psm.tile([1, CHUNKS], F32, tag="psmisc")
                        nc.tensor.matmul(
                            ps_a[:, :], lhsT=ones_col[:, :], rhs=acts_p[:, :],
                            start=True, stop=True,
                        )
                        cc_sb = tailp.tile([1, CHUNKS], F32, tag="cc_sb")
                        nc.scalar.copy(cc_sb[:, :], ps_a[:, :])
                        nc.sync.dma_start(cc_in[:, :], cc_sb[:, :])
                        nc.gpsimd.collective_compute(
                            "AllReduce",
                            OP.add,
                            replica_groups=[list(range(N_CORES))],
                            ins=[cc_in.opt()],
                            outs=[cc_out.opt()],
                        )

                    # transpose all 32 [128,128] blocks: x[b, f] -> xT[f, b]
                    xt = xtp.tile([P, F], BF16, tag="xt")
                    for g in range(2):
                        ps = pstr.tile([P, 16 * P], BF16, tag="ps_tr")
                        for j in range(16):
                            k = 16 * g + j
                            nc.tensor.transpose(
                                ps[:, j * P:(j + 1) * P],
                                xb[:, k * P:(k + 1) * P],
                                id_bf[:, :],
                            )
                        nc.scalar.copy(
                            xt[:, g * 16 * P:(g + 1) * 16 * P], ps[:, :]
                        )

                    # project every chunk through both head variants:
                    # psy_t[b, cv*10 + o] accumulated over the 2 halves
                    psy_t = psy.tile([P, W2C], F32, tag="psy")
                    for c in range(CHUNKS):
                        for h in range(2):
                            kh = 2 * c + h
                            nc.tensor.matmul(
                                psy_t[:, c * NV * OUT:(c + 1) * NV * OUT],
                                lhsT=xt[:, kh * P:(kh + 1) * P],
                                rhs=w2_sb[:, h * W2C + c * NV * OUT:
                                          h * W2C + (c + 1) * NV * OUT],
                                start=(h == 0),
                                stop=(h == 1),
                            )
                    # PSUM -> SBUF with (cv, o) -> (o, cv) layout swap so the
                    # tail reduce runs on a contiguous axis; f32 -> bf16 cast
                    nc.scalar.copy(
                        z_all[:, t * W2C:(t + 1) * W2C].rearrange(
                            "p (o cv) -> p o cv", cv=CV
                        ),
                        psy_t[:, :].rearrange("p (cv o) -> p o cv", o=OUT),
                    )

            # ---- tail: S -> top-4 mask -> fused combine -> store ----
            S = tailp.tile([1, CHUNKS], F32, tag="S")
            nc.sync.dma_start(S[:, :], cc_out[:, :])

            # top-4 threshold via 4x (max + mask-out); all on partition 0
            cur = tailp.tile([1, CHUNKS], F32, tag="cur")
            nc.vector.tensor_copy(cur[:, :], S[:, :])
            m1 = None
            mk = None
            for k in range(TOPK):
                mk = tailp.tile([1, 1], F32, tag=f"mk{k}")
                nc.vector.tensor_reduce(mk[:, :], cur[:, :], axis=AX.X, op=OP.max)
                if k == 0:
                    m1 = mk
                if k < TOPK - 1:
                    sel = tailp.tile([1, CHUNKS], F32, tag="sel")
                    # sel = (cur >= mk) * BIG_NEG  in one fused op
                    nc.vector.tensor_scalar(
                        sel[:, :], cur[:, :], mk[:, :], BIG_NEG,
                        op0=OP.is_ge, op1=OP.mult,
                    )
                    nc.vector.tensor_tensor(cur[:, :], cur[:, :], sel[:, :], op=OP.add)
            m4 = mk  # 4th largest

            mask16 = tailp.tile([1, CHUNKS], F32, tag="mask16")
            nc.vector.tensor_scalar(
                mask16[:, :], S[:, :], m4[:, :], None, op0=OP.is_ge
            )
            cond = tailp.tile([1, 1], F32, tag="cond")
            nc.vector.tensor_scalar(
                cond[:, :], m1[:, :], float(SUM_THRESH), None, op0=OP.is_gt
            )

            # combine row: [0:32) per-(c,v) weights, [32:42) bias(cond)
            BROW = CV + OUT
            brow = tailp.tile([1, BROW], F32, tag="brow")
            brow_cv = brow[:, 0:CV].rearrange("a (c v) -> a c v", v=NV)
            nc.vector.tensor_copy(brow_cv[:, :, 0:1], mask16[:, :].unsqueeze(2))
            nc.vector.tensor_scalar(
                brow_cv[:, :, 1:2], mask16[:, :].unsqueeze(2), cond[:, :],
                None, op0=OP.mult,
            )
            nc.vector.tensor_scalar(
                brow[:, CV:BROW], bd_row[:, :], cond[:, :], None, op0=OP.mult
            )
            nc.vector.tensor_tensor(
                brow[:, CV:BROW], brow[:, CV:BROW], bb_row[:, :], op=OP.add
            )

            # broadcast row -> all 128 partitions via K=1 matmul
            ps_b = psm.tile([P, BROW], F32, tag="psmisc")
            nc.tensor.matmul(
                ps_b[:, :], lhsT=ones_row[:, :], rhs=brow[:, :],
                start=True, stop=True,
            )
            bc32 = tailp.tile([P, CV], BF16, tag="bc32")
            nc.scalar.copy(bc32[:, :], ps_b[:, 0:CV])
            bias_bc = tailp.tile([P, OUT], F32, tag="bias_bc")
            nc.scalar.copy(bias_bc[:, :], ps_b[:, CV:BROW])

            # fused masked combine (DVE runs ~0.93 cyc/elem with bf16
            # outputs, ~1.5-1.8 with f32 outputs or reduces): bf16 mult,
            # one bf16 halving add, then a f32 reduce over the final 16
            tmp = tailp.tile([P, TILES * W2C], BF16, tag="tmp")
            nc.vector.tensor_tensor(
                tmp[:, :].rearrange("p (t o cv) -> p t o cv", o=OUT, cv=CV),
                z_all[:, :].rearrange("p (t o cv) -> p t o cv", o=OUT, cv=CV),
                bc32[:, :].unsqueeze(1).unsqueeze(2).broadcast_to(
                    [P, TILES, OUT, CV]
                ),
                op=OP.mult,
            )
            tv = tmp[:, :].rearrange("p (t o cv) -> p t o cv", o=OUT, cv=CV)
            nc.vector.tensor_tensor(
                tv[:, :, :, 0:CV // 2], tv[:, :, :, 0:CV // 2],
                tv[:, :, :, CV // 2:CV], op=OP.add,
            )
            out_sb = tailp.tile([P, TILES * OUT], F32, tag="out_sb")
            nc.vector.tensor_reduce(
                out_sb[:, :].rearrange("p (t o) -> p t o", o=OUT),
                tv[:, :, :, 0:CV // 2],
                axis=AX.X,
                op=OP.add,
            )
            nc.vector.tensor_tensor(
                out_sb[:, :].rearrange("p (t o) -> p t o", o=OUT),
                out_sb[:, :].rearrange("p (t o) -> p t o", o=OUT),
                bias_bc[:, :].unsqueeze(1).broadcast_to([P, TILES, OUT]),
                op=OP.add,
            )

            # store both halves in parallel on the two HWDGE rings
            half = TILES // 2
            nc.sync.dma_start(
                out_d.ap()[0:half * P, :].rearrange("(t p) o -> p t o", p=P),
                out_sb[:, 0:half * OUT].rearrange("p (t o) -> p t o", o=OUT),
            )
            nc.scalar.dma_start(
                out_d.ap()[half * P:TILES * P, :].rearrange(
                    "(t p) o -> p t o", p=P
                ),
                out_sb[:, half * OUT:TILES * OUT].rearrange(
                    "p (t o) -> p t o", o=OUT
                ),
            )

    nc.compile()
    return nc


def _get_nc():
    if "nc" not in _CACHE:
        _CACHE["nc"] = _build()
    return _CACHE["nc"]


def _prep_weights(expert_w, expert_b, quant_w, quant_b):
    import ml_dtypes

    qmean = np.float32(np.mean(np.abs(quant_w)))
    wbin = (np.sign(quant_w) * qmean).astype(np.float32)       # [o', o]
    d = (quant_w - wbin).astype(np.float32)
    # fold head variants into expert weights: w2[c, v, o', f]
    w2_bin = np.einsum("po,cof->cpf", wbin, expert_w)
    w2_d = np.einsum("po,cof->cpf", d, expert_w)
    w2 = np.stack([w2_bin, w2_d], axis=1)                      # [16, 2, 10, 256]
    wr = w2.reshape(CHUNKS, NV, OUT, 2, P)                     # c, v, o, h, p
    w2_sb = np.ascontiguousarray(
        wr.transpose(4, 3, 0, 1, 2).reshape(P, 2 * W2C)
    ).astype(ml_dtypes.bfloat16)
    bias_b0 = (expert_b @ wbin.T + quant_b).reshape(1, OUT).astype(np.float32)
    bias_d = (expert_b @ d.T).reshape(1, OUT).astype(np.float32)
    id_bf = np.eye(P, dtype=ml_dtypes.bfloat16)
    return w2_sb, bias_b0, bias_d, id_bf


def kernel(x, expert_w, expert_b, quant_w, quant_b):
    x = np.ascontiguousarray(np.asarray(x, dtype=np.float32))
    expert_w = np.asarray(expert_w, dtype=np.float32)
    expert_b = np.asarray(expert_b, dtype=np.float32)
    quant_w = np.asarray(quant_w, dtype=np.float32)
    quant_b = np.asarray(quant_b, dtype=np.float32)

    w2_sb, bias_b0, bias_d, id_bf = _prep_weights(
        expert_w, expert_b, quant_w, quant_b
    )
    nc = _get_nc()
    in_maps = []
    for i in range(N_CORES):
        in_maps.append({
            "x": np.ascontiguousarray(x[i * BC:(i + 1) * BC]),
            "w2_sb": w2_sb,
            "bias_b0": bias_b0,
            "bias_d": bias_d,
            "id_bf": id_bf,
        })

    res = run_bass_kernel_spmd(nc, in_maps, core_ids=list(range(N_CORES)))
    out = np.concatenate(
        [np.asarray(res.results[i]["out"]) for i in range(N_CORES)], axis=0
    )
    return out.astype(np.float32)
